# revision 7
# baseline (speedup 1.0000x reference)
"""HGT (heterogeneous graph transformer) Bass kernel for 8 TRN2 NeuronCores.

Strategy: destination-sharded edge processing. Host bin-packs each node type's
nodes into 208 windows (26/core) of <=128 slots and exactly 1280 edges; the
segment-softmax + segment-sum for a window is done fully on one core via
one-hot matmuls on the TensorEngine (scatter = S_T^T @ [msg|escore] with PSUM
accumulation; q-expand = S2^T @ q_window).  k/v tables are built node-sharded
and exchanged with one AllGather per layer; per-edge k/v rows are fetched with
dma_gather (512B rows, bf16).  Softmax is computed without the running max
(scores are small; exp is safe in fp32) so normalization commutes with the
segment sum and needs no second pass over edges.
"""

import os
import numpy as np
import ml_dtypes

import concourse.bass as bass
import concourse.tile as tile
from concourse import bacc, mybir
from concourse import bass_utils

H, D, HID, OUT = 8, 16, 128, 64
N, E, L, T, R = 25000, 250000, 2, 2, 2
NCORE = 8
NWC = 26                 # windows per core
NWIN = NCORE * NWC       # 208 windows per node type
WE = 1280                # edges per window (padded)
CPW = WE // 128          # 10 chunks per window
CHUNKS = NWC * CPW       # 260 chunks per core per relation
EPC = CHUNKS * 128       # 33280 edges per core per relation
SCC = 5                  # chunks per superchunk
NSC = CHUNKS // SCC      # 52
CN = NWC * 128           # 3328 rows per core per type
ROWS = NWIN * 128        # 26624 table rows per type
FP8_ONE = 0x38           # fp8e4m3 1.0
# relation -> (src_type, dst_type): rel0 = writes author->paper, rel1 = rev
REL_SRC = [0, 1]
REL_DST = [1, 0]

f32 = mybir.dt.float32
bf16 = mybir.dt.bfloat16
fp8 = mybir.dt.float8e4
i16 = mybir.dt.int16
BF = ml_dtypes.bfloat16


def _binpack(deg):
    """node -> global row (win*128+slot), snake-dealt by descending degree."""
    order = np.argsort(-deg, kind="stable")
    k = np.arange(N)
    pos = k % (2 * NWIN)
    bins = np.where(pos < NWIN, pos, 2 * NWIN - 1 - pos)
    srt = np.argsort(bins, kind="stable")
    sb = bins[srt]
    start = np.searchsorted(sb, np.arange(NWIN), side="left")
    slotv = np.arange(N) - start[sb]
    slot = np.empty(N, np.int64)
    slot[srt] = slotv
    assert slot.max() <= 126, slot.max()
    row = np.empty(N, np.int64)
    row[order] = bins * 128 + slot
    # check per-window edge capacity
    wsum = np.zeros(NWIN, np.int64)
    np.add.at(wsum, row // 128, deg)
    assert wsum.max() <= WE, f"window edge overflow {wsum.max()}"
    return row


def _edge_arrays(ei, row_src, row_dst):
    """Per-core padded edge arrays: src table rows + dst slots, window-major."""
    src, dst = ei[0].astype(np.int64), ei[1].astype(np.int64)
    drow = row_dst[dst]
    win = drow // 128
    slot = drow % 128
    o = np.argsort(win, kind="stable")
    wins, slots, srows = win[o], slot[o], row_src[src[o]]
    counts = np.bincount(wins, minlength=NWIN)
    starts = np.concatenate([[0], np.cumsum(counts)[:-1]])
    posw = np.arange(E) - starts[wins]
    src_p = np.zeros((NWIN, WE), np.int64)       # pad: src row 0
    slot_p = np.full((NWIN, WE), 127, np.int64)  # pad: trash slot
    src_p[wins, posw] = srows
    slot_p[wins, posw] = slots
    return src_p, slot_p


def _mk_S(slot_flat):
    """slot_flat [EPC] -> (S_T [128,CHUNKS,128] u8, S2 [128,CHUNKS,128] u8)."""
    s = slot_flat.reshape(CHUNKS, 128)
    A = np.zeros((CHUNKS, 128, 128), np.uint8)
    np.put_along_axis(A, s[:, :, None], FP8_ONE, axis=2)
    S_T = np.ascontiguousarray(A.transpose(1, 0, 2))      # [e_part, chunk, slot]
    S2 = np.ascontiguousarray(A.transpose(2, 0, 1))       # [slot_part, chunk, e]
    return S_T, S2


def _wrap_idx(idx):
    """[EPC] int -> [16, EPC//16] int16 gather layout (i%16, i//16)."""
    return np.ascontiguousarray(np.tile(idx.reshape(-1, 16).T.astype(np.int16), (8, 1)))


def _build_program(nc, tc, consts):
    betas = consts["betas"]  # [L][T] python floats
    dram = {}

    def din(name, shape, dt):
        t = nc.dram_tensor(name, list(shape), dt, kind="ExternalInput")
        dram[name] = t
        return t

    xcm = [din(f"xcm{t}", [128, CN], f32) for t in range(T)]
    w_in = [din(f"w_in{t}", [128, 128], f32) for t in range(T)]
    b_in = [din(f"b_in{t}", [1, 128], f32) for t in range(T)]
    wk = [[din(f"wk{l}{t}", [128, 128], f32) for t in range(T)] for l in range(L)]
    bk = [[din(f"bk{l}{t}", [1, 128], f32) for t in range(T)] for l in range(L)]
    wv = [[din(f"wv{l}{t}", [128, 128], f32) for t in range(T)] for l in range(L)]
    bv = [[din(f"bv{l}{t}", [1, 128], f32) for t in range(T)] for l in range(L)]
    wq = [[din(f"wq{l}{r}", [128, 128], f32) for r in range(R)] for l in range(L)]
    bq = [[din(f"bq{l}{r}", [1, 128], f32) for r in range(R)] for l in range(L)]
    bdm = [[din(f"bdm{l}{r}", [128, 128], f32) for r in range(R)] for l in range(L)]
    wa = [[din(f"wa{l}{t}", [128, 128], f32) for t in range(T)] for l in range(L)]
    ba = [[din(f"ba{l}{t}", [1, 128], f32) for t in range(T)] for l in range(L)]
    wo = din("wo", [128, 64], f32)
    bo = din("bo", [1, 64], f32)
    ones_d = din("ones", [1, 512], f32)
    ident_d = din("ident", [128, 128], f32)
    st_d = [din(f"st{r}", [128, CHUNKS * 128], fp8) for r in range(R)]
    s2_d = [din(f"s2{r}", [128, CHUNKS * 128], fp8) for r in range(R)]
    idx_d = [din(f"idx{r}", [128, EPC // 16], i16) for r in range(R)]

    tab = [nc.dram_tensor(f"tab{l}", [ROWS, 512], bf16, addr_space="Shared")
           for l in range(L)]
    agin = [nc.dram_tensor(f"agin{l}", [CN, 512], bf16) for l in range(L)]
    outs = [nc.dram_tensor(f"o{t}", [NWC, 64, 128], f32, kind="ExternalOutput")
            for t in range(T)]

    es = consts["es"]
    persist = es.enter_context(tc.tile_pool(name="persist", bufs=1))
    wpool = es.enter_context(tc.tile_pool(name="wts", bufs=1))
    sb = es.enter_context(tc.tile_pool(name="work", bufs=3))
    spool = es.enter_context(tc.tile_pool(name="smats", bufs=3))
    kvpool = es.enter_context(tc.tile_pool(name="kvg", bufs=3))
    pq = es.enter_context(tc.tile_pool(name="pq", bufs=1, space="PSUM"))
    pw = es.enter_context(tc.tile_pool(name="pw", bufs=2, space="PSUM"))
    pm = es.enter_context(tc.tile_pool(name="pm", bufs=3, space="PSUM"))

    def load_const(dt_, dten, shape):
        t_ = wpool.tile(list(shape), dt_, tag=dten.name)
        nc.sync.dma_start(out=t_[:], in_=dten[:])
        return t_

    ones = load_const(f32, ones_d, [1, 512])
    ident = load_const(f32, ident_d, [128, 128])
    ws = {}
    for nm, dten in dram.items():
        if nm.startswith(("w_in", "b_in", "wk", "bk", "wv", "bv", "wq", "bq",
                          "bdm", "wa", "ba", "wo", "bo")):
            ws[nm] = load_const(f32, dten, dten.shape)

    # persistent activations
    xs = [persist.tile([128, CN], f32, tag=f"xs{t}", name=f"xs{t}") for t in range(T)]
    nx = [persist.tile([128, CN], f32, tag=f"nx{t}", name=f"nx{t}") for t in range(T)]
    qt = [persist.tile([128, CN], bf16, tag=f"qt{r}", name=f"qt{r}") for r in range(R)]
    idxsb = [persist.tile([128, EPC // 16], i16, tag=f"idx{r}", name=f"idxs{r}") for r in range(R)]
    for r in range(R):
        nc.sync.dma_start(out=idxsb[r][:], in_=idx_d[r][:])

    # ---- layer 0 input projection (channel-major): xs = relu(W_in^T x + b)
    for t in range(T):
        xr = sb.tile([128, CN], f32, tag="xraw")
        nc.sync.dma_start(out=xr[:], in_=xcm[t][:])
        for s0 in range(0, CN, 512):
            sl = min(512, CN - s0)
            ps = pq.tile([128, 640], f32, tag="qd")
            nc.tensor.matmul(ps[:, 0:sl], ws[f"w_in{t}"][:], xr[:, s0:s0 + sl],
                             start=True, stop=False)
            nc.tensor.matmul(ps[:, 0:sl], ws[f"b_in{t}"][:], ones[:, 0:sl],
                             start=False, stop=True)
            nc.scalar.activation(xs[t][:, s0:s0 + sl], ps[:, 0:sl],
                                 mybir.ActivationFunctionType.Relu)

    for l in range(L):
        cur = xs if l == 0 else nx
        # ---- build own k/v table slice (node-major rows) + stage to agin
        for t in range(T):
            for w in range(NWC):
                ps = pm.tile([128, 256], f32, tag="bld")
                xw = cur[t][:, w * 128:(w + 1) * 128]
                nc.tensor.matmul(ps[:, 0:128], xw, ws[f"wk{l}{t}"][:],
                                 start=True, stop=False)
                nc.tensor.matmul(ps[:, 0:128], ones[:, 0:128], ws[f"bk{l}{t}"][:],
                                 start=False, stop=True)
                nc.tensor.matmul(ps[:, 128:256], xw, ws[f"wv{l}{t}"][:],
                                 start=True, stop=False)
                nc.tensor.matmul(ps[:, 128:256], ones[:, 0:128], ws[f"bv{l}{t}"][:],
                                 start=False, stop=True)
                stg = sb.tile([128, 256], bf16, tag="stg")
                nc.scalar.activation(stg[:], ps[:],
                                     mybir.ActivationFunctionType.Copy)
                nc.sync.dma_start(
                    out=agin[l][w * 128:(w + 1) * 128, t * 256:(t + 1) * 256],
                    in_=stg[:])
        # ---- q~ tables (own dst windows, node-major, bf16)
        for r in range(R):
            td = REL_DST[r]
            for w in range(NWC):
                ps = pm.tile([128, 256], f32, tag="bld")
                xw = cur[td][:, w * 128:(w + 1) * 128]
                nc.tensor.matmul(ps[:, 0:128], xw, ws[f"wq{l}{r}"][:],
                                 start=True, stop=False)
                nc.tensor.matmul(ps[:, 0:128], ones[:, 0:128], ws[f"bq{l}{r}"][:],
                                 start=False, stop=True)
                nc.scalar.activation(qt[r][:, w * 128:(w + 1) * 128], ps[:, 0:128],
                                     mybir.ActivationFunctionType.Copy)
        # ---- exchange tables
        nc.gpsimd.collective_compute(
            "AllGather", mybir.AluOpType.bypass,
            replica_groups=[list(range(NCORE))],
            ins=[agin[l].ap().opt()], outs=[tab[l].ap().opt()])
        # ---- prescale residual
        dst = xs if l == 0 else nx     # in-place
        for t in range(T):
            nc.vector.tensor_scalar_mul(dst[t][:], dst[t][:],
                                        float(1.0 - betas[l][t]))
        nxt = nx if l == 0 else xs     # write next activations into other buf
        # ---- edge phase
        for r in range(R if not int(os.environ.get('K_SKIP_EDGE','0')) else 0):
            ts, td = REL_SRC[r], REL_DST[r]
            for sc in range(int(os.environ.get('K_NSC', NSC))):
                kv = kvpool.tile([128, SCC, 256], bf16, tag="kv")
                nc.gpsimd.dma_gather(
                    out_ap=kv[:],
                    in_ap=tab[l][:, ts * 256:(ts + 1) * 256],
                    idxs_ap=idxsb[r][:, sc * 40:(sc + 1) * 40],
                    num_idxs=SCC * 128, num_idxs_reg=SCC * 128,
                    elem_size=256, elem_step=512)
                s2t = spool.tile([128, SCC * 128], fp8, tag="s2")
                stt = spool.tile([128, SCC * 128], fp8, tag="st")
                nc.sync.dma_start(
                    out=s2t[:], in_=s2_d[r][:, sc * SCC * 128:(sc + 1) * SCC * 128])
                nc.sync.dma_start(
                    out=stt[:], in_=st_d[r][:, sc * SCC * 128:(sc + 1) * SCC * 128])
                qdp = pq.tile([128, 640], f32, tag="qd")
                for j in range(SCC):
                    g = sc * SCC + j
                    w = g // CPW
                    nc.tensor.matmul(qdp[:, j * 128:(j + 1) * 128],
                                     s2t[:, j * 128:(j + 1) * 128],
                                     qt[r][:, w * 128:(w + 1) * 128],
                                     start=True, stop=True)
                qds = sb.tile([128, 640], bf16, tag="qds")
                nc.scalar.activation(qds[:], qdp[:],
                                     mybir.ActivationFunctionType.Copy)
                qk = sb.tile([128, SCC, 128], bf16, tag="qk")
                nc.vector.tensor_tensor(
                    out=qk[:], in0=qds[:].rearrange("p (c e) -> p c e", c=SCC),
                    in1=kv[:, :, 0:128], op=mybir.AluOpType.mult)
                scf = sb.tile([128, 40], f32, tag="scf")
                nc.vector.tensor_reduce(
                    out=scf[:], in_=qk[:].rearrange("p c (h d) -> p (c h) d", d=D),
                    axis=mybir.AxisListType.X, op=mybir.AluOpType.add)
                esc = sb.tile([128, 40], bf16, tag="esc")
                nc.scalar.activation(esc[:], scf[:],
                                     mybir.ActivationFunctionType.Exp)
                me = sb.tile([128, SCC, 136], bf16, tag="me")
                nc.scalar.activation(
                    me[:, :, 128:136], scf[:].rearrange("p (c h) -> p c h", c=SCC),
                    mybir.ActivationFunctionType.Exp)
                for j in range(SCC):
                    g = sc * SCC + j
                    nc.gpsimd.tensor_tensor(
                        out=me[:, j, 0:128].rearrange("p (h d) -> p h d", d=D),
                        in0=kv[:, j, 128:256].rearrange("p (h d) -> p h d", d=D),
                        in1=esc[:, j * 8:(j + 1) * 8]
                            .rearrange("p (h o) -> p h o", o=1)
                            .broadcast_to([128, 8, D]),
                        op=mybir.AluOpType.mult)
                for j in range(SCC):
                    g = sc * SCC + j
                    w = g // CPW
                    if g % CPW == 0:
                        wps = pw.tile([128, 136], f32, tag="wps")
                    nc.tensor.matmul(wps[:], stt[:, j * 128:(j + 1) * 128],
                                     me[:, j, :], start=(g % CPW == 0),
                                     stop=(g % CPW == CPW - 1),
                                     skip_group_check=True)
                    if g % CPW == CPW - 1:
                        den = sb.tile([128, 8], f32, tag="den")
                        nc.vector.tensor_scalar_add(den[:], wps[:, 128:136], 1e-16)
                        rec = sb.tile([128, 8], f32, tag="rec")
                        nc.vector.reciprocal(rec[:], den[:])
                        agn = sb.tile([128, 128], f32, tag="agn")
                        nc.vector.tensor_tensor(
                            out=agn[:].rearrange("p (h d) -> p h d", d=D),
                            in0=wps[:, 0:128].rearrange("p (h d) -> p h d", d=D),
                            in1=rec[:].rearrange("p (h o) -> p h o", o=1)
                                .broadcast_to([128, 8, D]),
                            op=mybir.AluOpType.mult)
                        tps = pm.tile([128, 256], f32, tag="bld")
                        nc.tensor.transpose(tps[:, 0:128], agn[:], ident[:])
                        agc = sb.tile([128, 128], f32, tag="agc")
                        nc.scalar.activation(agc[:], tps[:, 0:128],
                                             mybir.ActivationFunctionType.Copy)
                        bps = pm.tile([128, 256], f32, tag="bld")
                        nc.tensor.matmul(bps[:, 0:128], ws[f"bdm{l}{r}"][:],
                                         agc[:], start=True, stop=True)
                        gel = sb.tile([128, 128], f32, tag="gel")
                        nc.scalar.activation(gel[:], bps[:, 0:128],
                                             mybir.ActivationFunctionType.Gelu)
                        ops4 = pm.tile([128, 256], f32, tag="bld")
                        nc.tensor.matmul(ops4[:, 0:128], ws[f"wa{l}{td}"][:],
                                         gel[:], start=True, stop=False)
                        nc.tensor.matmul(ops4[:, 0:128], ws[f"ba{l}{td}"][:],
                                         ones[:, 0:128], start=False, stop=True)
                        nxw = sb.tile([128, 128], f32, tag="nxw")
                        nc.scalar.activation(nxw[:], ops4[:, 0:128],
                                             mybir.ActivationFunctionType.Copy,
                                             scale=float(betas[l][td]))
                        nc.vector.tensor_tensor(
                            out=nxt[td][:, w * 128:(w + 1) * 128], in0=nxw[:],
                            in1=dst[td][:, w * 128:(w + 1) * 128],
                            op=mybir.AluOpType.add)
    # ---- output projection (channel-major out [64, n])
    fin = xs if L % 2 == 0 else nx
    for t in range(T):
        for w in range(NWC):
            ps = pm.tile([128, 256], f32, tag="bld")
            nc.tensor.matmul(ps[0:64, 0:128], ws["wo"][:],
                             fin[t][:, w * 128:(w + 1) * 128],
                             start=True, stop=False)
            nc.tensor.matmul(ps[0:64, 0:128], ws["bo"][:], ones[:, 0:128],
                             start=False, stop=True)
            ot = sb.tile([64, 128], f32, tag="ot")
            nc.scalar.activation(ot[:], ps[0:64, 0:128],
                                 mybir.ActivationFunctionType.Copy)
            nc.sync.dma_start(out=outs[t][w], in_=ot[:])


def kernel(**inputs):
    x = [np.asarray(inputs["x_author"], np.float32),
         np.asarray(inputs["x_paper"], np.float32)]
    eis = [np.asarray(inputs["edge_writes"]), np.asarray(inputs["edge_rev"])]
    W_in = np.asarray(inputs["W_in"], np.float32)
    b_in = np.asarray(inputs["b_in"], np.float32)
    Wq, bq_ = np.asarray(inputs["Wq"], np.float32), np.asarray(inputs["bq"], np.float32)
    Wk, bk_ = np.asarray(inputs["Wk"], np.float32), np.asarray(inputs["bk"], np.float32)
    Wv, bv_ = np.asarray(inputs["Wv"], np.float32), np.asarray(inputs["bv"], np.float32)
    Wa, ba_ = np.asarray(inputs["Wa"], np.float32), np.asarray(inputs["ba"], np.float32)
    skip = np.asarray(inputs["skip"], np.float32)
    a_rel = np.asarray(inputs["a_rel"], np.float32)
    m_rel = np.asarray(inputs["m_rel"], np.float32)
    p_rel = np.asarray(inputs["p_rel"], np.float32)
    W_out = np.asarray(inputs["W_out"], np.float32)
    b_out = np.asarray(inputs["b_out"], np.float32)

    # ---- host sharding prep
    deg = [np.bincount(eis[r][1].astype(np.int64), minlength=N) for r in range(R)]
    # node type t is dst of relation rel_of_dst[t]: type0(author)<-rel1, type1<-rel0
    row = [None, None]
    row[0] = _binpack(deg[1])
    row[1] = _binpack(deg[0])
    edge = [_edge_arrays(eis[r], row[REL_SRC[r]], row[REL_DST[r]]) for r in range(R)]

    betas = [[float(1.0 / (1.0 + np.exp(-skip[l, t]))) for t in range(T)]
             for l in range(L)]
    invsD = 1.0 / np.sqrt(D)

    def bd(mats):  # [H,16,16] -> blockdiag [128,128]
        o = np.zeros((HID, HID), np.float32)
        for h in range(H):
            o[h * D:(h + 1) * D, h * D:(h + 1) * D] = mats[h]
        return o

    common = {"wo": W_out, "bo": b_out[None, :], "ones": np.ones((1, 512), np.float32),
              "ident": np.eye(128, dtype=np.float32)}
    for t in range(T):
        common[f"w_in{t}"] = W_in[t]
        common[f"b_in{t}"] = b_in[t][None, :]
    for l in range(L):
        for t in range(T):
            common[f"wk{l}{t}"] = Wk[l, t]
            common[f"bk{l}{t}"] = bk_[l, t][None, :]
            common[f"wv{l}{t}"] = Wv[l, t]
            common[f"bv{l}{t}"] = bv_[l, t][None, :]
            common[f"wa{l}{t}"] = Wa[l, t]
            common[f"ba{l}{t}"] = ba_[l, t][None, :]
        for r in range(R):
            td = REL_DST[r]
            BDq = bd([a_rel[l, r, h].T for h in range(H)])
            scl = np.repeat(p_rel[l, r] * invsD, D)[None, :].astype(np.float32)
            common[f"wq{l}{r}"] = (Wq[l, td] @ BDq) * scl
            common[f"bq{l}{r}"] = (bq_[l, td] @ BDq * scl[0])[None, :]
            common[f"bdm{l}{r}"] = bd([m_rel[l, r, h] for h in range(H)])

    in_maps = []
    for c in range(NCORE):
        m = {k: np.ascontiguousarray(v, np.float32) for k, v in common.items()}
        for t in range(T):
            xp = np.zeros((CN, HID), np.float32)
            sel = (row[t] // 128 >= c * NWC) & (row[t] // 128 < (c + 1) * NWC)
            loc = row[t][sel] - c * CN
            xp[loc] = x[t][sel]
            m[f"xcm{t}"] = np.ascontiguousarray(xp.T)
        for r in range(R):
            src_p, slot_p = edge[r]
            sl = slice(c * NWC, (c + 1) * NWC)
            srows = src_p[sl].reshape(-1)
            slots = slot_p[sl].reshape(-1)
            S_T, S2 = _mk_S(slots)
            m[f"st{r}"] = S_T.reshape(128, CHUNKS * 128)
            m[f"s2{r}"] = S2.reshape(128, CHUNKS * 128)
            m[f"idx{r}"] = _wrap_idx(srows)
        in_maps.append(m)

    nc = bacc.Bacc("TRN2", target_bir_lowering=False, debug=False,
                   enable_asserts=False, num_devices=NCORE)
    from contextlib import ExitStack
    with tile.TileContext(nc, trace_sim=False) as tc:
        with ExitStack() as es:
            _build_program(nc, tc, {"betas": betas, "es": es})
    nc.compile()

    res = bass_utils.run_bass_kernel_spmd(nc, in_maps, core_ids=list(range(NCORE)))
    global LAST_EXEC_NS, LAST_RES
    LAST_EXEC_NS = res.exec_time_ns
    LAST_RES = res
    if int(os.environ.get("K_TIME", "0")):
        import time as _time
        ts = []
        for _ in range(3):
            t0 = _time.perf_counter()
            bass_utils.run_bass_kernel_spmd(nc, in_maps, core_ids=list(range(NCORE)))
            ts.append(_time.perf_counter() - t0)
        LAST_EXEC_NS = int(min(ts) * 1e9)
    outs = []
    for t in range(T):
        big = np.zeros((NCORE * CN, OUT), np.float32)
        for c in range(NCORE):
            o = np.asarray(res.results[c][f"o{t}"])      # [NWC, 64, 128]
            big[c * CN:(c + 1) * CN] = o.transpose(0, 2, 1).reshape(CN, OUT)
        outs.append(big[row[t]])
    return outs[0], outs[1]
